# revision 16
# baseline (speedup 1.0000x reference)
"""BjorckLinear TRN2 kernel (8-core SPMD, data-parallel over batch).

reference semantics:
    w10 = bjorck_orthonormalize(weight)   # exactly 10 order-1 iterations
    out = inputs @ w10.T

For this problem's input distribution (sigma_min(W0) ~ 2e-4) the
reference's early-stop (max|dW| <= 1e-6) never fires before the 10-iter
cap, so a fixed 10-iteration loop reproduces the reference dynamics.

Device algorithm per core (matmuls in float32r; scaling exact in f32):
    iterate W (with WT = W^T maintained via PE transposes):
        S = W^T W               (lhsT = W chunks, rhs = W)
        G = S - 3I              (DVE/ACT eviction + diagonal-block subtract)
        W' = -0.5 * (W G)       (lhsT = WT, rhs = G; -0.5 in the eviction)
        WT' = transpose(W')     (PE transpose, 128x128 blocks, sub-major
                                 order so each transpose only waits on one
                                 just-evicted W' chunk)
    which equals W' = 1.5 W - 0.5 W (W^T W).
    After 10 iterations V10 = WT10 = W10^T, then Yt = W10 @ Xt with
    lhsT = V10 chunks, rhs = Xt tiles streamed from HBM.

Sharding: weight + Bjorck replicated on all 8 cores; `inputs` split
along batch into 8 shards of 16384 rows, passed host-transposed as
Xt = [512, 16384] so both matmul operands are contraction-major.
Output comes back as Yt = [512, 16384] per core, host-untransposed.

Engine plan: PE matmuls; DVE evicts PSUM (plus diagonal fixups); ACT
helps with Bjorck evictions and issues the 1MB y-out DMAs on its own
HWDGE ring so output flow cannot head-of-line-block the x-in stream
on Sync's ring.
"""
import numpy as np

import concourse.bacc as bacc
import concourse.mybir as mybir
import concourse.tile as tile
from concourse.bass_utils import run_bass_kernel_spmd

dt = mybir.dt

P = 128
D = 512
KC = D // P            # 4 contraction chunks
ITERS = 10
N_CORES = 8
BATCH = 131072
SHARD = BATCH // N_CORES   # 16384

XBLK = 2048            # batch columns per x super-block
NXB = SHARD // XBLK    # 8 super-blocks
NSUB = XBLK // 512     # 4 matmul sub-blocks (N=512) per super-block
XBUFS = 4
YBLK = 2048
YBUFS = 2

PSUM_TAGS = ["pa", "pb", "pc", "pd"]


def build():
    nc = bacc.Bacc("TRN2", target_bir_lowering=False, debug=False)
    # float32r dram views: same bits as float32; PE rounds internally.
    xt_dram = nc.dram_tensor("xt", [D, SHARD], dt.float32r, kind="ExternalInput")
    w_dram = nc.dram_tensor("w", [D, D], dt.float32r, kind="ExternalInput")
    wt_dram = nc.dram_tensor("wt", [D, D], dt.float32r, kind="ExternalInput")
    # e128 = 3 * I_128 (diagonal block of 3I lies in column slice mi of
    # row-chunk mi); i128 = I_128 for PE transposes.
    e_dram = nc.dram_tensor("e128", [P, P], dt.float32, kind="ExternalInput")
    i_dram = nc.dram_tensor("i128", [P, P], dt.float32r, kind="ExternalInput")
    yt_dram = nc.dram_tensor("yt", [D, SHARD], dt.float32, kind="ExternalOutput")

    with tile.TileContext(nc) as tc:
        with (
            tc.tile_pool(name="const", bufs=1) as const,
            tc.tile_pool(name="bj", bufs=2) as bj,
            tc.tile_pool(name="gp", bufs=1) as gp,
            tc.tile_pool(name="xp", bufs=XBUFS) as xp,
            tc.tile_pool(name="yp", bufs=YBUFS) as yp,
            tc.tile_pool(name="psum", bufs=2, space="PSUM") as psum,
        ):
            # ---------- Bjorck (replicated) ----------
            W = []
            for k in range(KC):
                wk = bj.tile([P, D], dt.float32r, tag=f"w_{k}")
                nc.sync.dma_start(wk[:], w_dram[k * P:(k + 1) * P, :])
                W.append(wk)
            WT = []
            for k in range(KC):
                vk = bj.tile([P, D], dt.float32r, tag=f"wt_{k}")
                nc.sync.dma_start(vk[:], wt_dram[k * P:(k + 1) * P, :])
                WT.append(vk)
            e128 = const.tile([P, P], dt.float32, tag="e128")
            nc.sync.dma_start(e128[:], e_dram[:, :])
            i128 = const.tile([P, P], dt.float32r, tag="i128")
            nc.sync.dma_start(i128[:], i_dram[:, :])

            for it in range(ITERS):
                last = it == ITERS - 1
                # S = W^T W ; G = S - 3I   (S groups on tags pa/pb)
                G = []
                for mi in range(KC):
                    msl = slice(mi * P, (mi + 1) * P)
                    ps = psum.tile([P, D], dt.float32, tag=PSUM_TAGS[mi % 2],
                                   name=f"ps_s_{it}_{mi}")
                    for ki in range(KC):
                        nc.tensor.matmul(ps[:], W[ki][:, msl], W[ki][:],
                                         start=(ki == 0), stop=(ki == KC - 1))
                    g = gp.tile([P, D], dt.float32r, tag=f"g_{mi}")
                    if mi < 2:
                        nc.scalar.copy(g[:], ps[:])
                    else:
                        nc.vector.tensor_copy(g[:], ps[:])
                    # diagonal block: G[:, msl] = S[:, msl] - 3I
                    nc.vector.tensor_tensor(g[:, msl], ps[:, msl], e128[:],
                                            mybir.AluOpType.subtract)
                    G.append(g)

                # W' = -0.5 * (W G), lhsT = WT   (tag pc)
                newW = []
                for mi in range(KC):
                    msl = slice(mi * P, (mi + 1) * P)
                    ps = psum.tile([P, D], dt.float32, tag="pc",
                                   name=f"ps_w_{it}_{mi}")
                    for ki in range(KC):
                        nc.tensor.matmul(ps[:], WT[ki][:, msl], G[ki][:],
                                         start=(ki == 0), stop=(ki == KC - 1))
                    wn = bj.tile([P, D], dt.float32r, tag=f"w_{mi}")
                    if mi < 2:
                        nc.scalar.mul(wn[:], ps[:], -0.5)
                    else:
                        nc.vector.tensor_scalar_mul(wn[:], ps[:], -0.5)
                    newW.append(wn)

                # WT' = transpose(W') via PE, mi-major through tag pd
                newWT = []
                for mi in range(KC):
                    tps = psum.tile([P, D], dt.float32r, tag="pd",
                                    name=f"ps_t_{it}_{mi}")
                    for sub in range(KC):
                        ssl = slice(sub * P, (sub + 1) * P)
                        nc.tensor.transpose(tps[:, ssl],
                                            newW[sub][:, mi * P:(mi + 1) * P],
                                            i128[:])
                    if last:
                        vt = const.tile([P, D], dt.float32r, tag=f"v10_{mi}")
                    else:
                        vt = bj.tile([P, D], dt.float32r, tag=f"wt_{mi}")
                    nc.vector.tensor_copy(vt[:], tps[:])
                    newWT.append(vt)
                W, WT = newW, newWT
            V10 = WT

            # ---------- linear: Yt = W10 @ Xt  (lhsT = V10) ----------
            # loop order reuses each V10 weight chunk across NSUB moving
            # blocks; psum tags per js give 4 live banks + double buffer.
            for nb in range(NXB):
                bsl = slice(nb * XBLK, (nb + 1) * XBLK)
                X = []
                for k in range(KC):
                    xk = xp.tile([P, XBLK], dt.float32r, tag=f"x_{k}",
                                 name=f"x_{nb}_{k}")
                    nc.sync.dma_start(xk[:], xt_dram[k * P:(k + 1) * P, bsl])
                    X.append(xk)
                for mi in range(KC):
                    msl = slice(mi * P, (mi + 1) * P)
                    PS = [psum.tile([P, 512], dt.float32, tag=PSUM_TAGS[js],
                                    name=f"ps_y_{nb}_{mi}_{js}")
                          for js in range(NSUB)]
                    yt = yp.tile([P, YBLK], dt.float32, tag="y",
                                 name=f"y_{nb}_{mi}")
                    for ki in range(KC):
                        for js in range(NSUB):
                            nc.tensor.matmul(
                                PS[js][:], V10[ki][:, msl],
                                X[ki][:, js * 512:(js + 1) * 512],
                                start=(ki == 0), stop=(ki == KC - 1))
                    for js in range(NSUB):
                        if js == 0:
                            nc.scalar.copy(yt[:, js * 512:(js + 1) * 512],
                                           PS[js][:])
                        else:
                            nc.vector.tensor_copy(
                                yt[:, js * 512:(js + 1) * 512], PS[js][:])
                    # y-out (1MB) on the Activation HWDGE ring, separate
                    # from the x-in stream on Sync's ring
                    nc.scalar.dma_start(
                        yt_dram[mi * P:(mi + 1) * P, bsl], yt[:])
    nc.compile()
    return nc


_CACHE = {}


def _get_nc():
    if "nc" not in _CACHE:
        _CACHE["nc"] = build()
    return _CACHE["nc"]


def make_in_maps(inputs, weight):
    w = np.ascontiguousarray(weight, dtype=np.float32)
    wt = np.ascontiguousarray(w.T)
    e128 = (3.0 * np.eye(P)).astype(np.float32)
    i128 = np.eye(P, dtype=np.float32)
    x = np.ascontiguousarray(inputs, dtype=np.float32)
    in_maps = []
    for c in range(N_CORES):
        xt_c = np.ascontiguousarray(x[c * SHARD:(c + 1) * SHARD, :].T)
        in_maps.append({"xt": xt_c, "w": w, "wt": wt,
                        "e128": e128, "i128": i128})
    return in_maps


def kernel(inputs: np.ndarray, weight: np.ndarray) -> np.ndarray:
    assert inputs.shape == (BATCH, D) and weight.shape == (D, D)
    nc = _get_nc()
    in_maps = make_in_maps(inputs, weight)
    res = run_bass_kernel_spmd(nc, in_maps, core_ids=list(range(N_CORES)))
    out = np.empty((BATCH, D), dtype=np.float32)
    for c in range(N_CORES):
        out[c * SHARD:(c + 1) * SHARD, :] = res.results[c]["yt"].T
    return out
